# revision 5
# baseline (speedup 1.0000x reference)
"""Trainium2 Bass kernel for nn_EuclideanCodebook (EnCodec VQ codebook, training step).

Data-parallel over 8 NeuronCores: flattened tokens N=32*4096=131072 are sharded
128 tiles/core x 128 tokens; embed (1024x128) is replicated. Per core:

  dist[t,k] = 2*x_t.e_k - |e_k|^2          (fp32 PE matmul + fused DVE subtract)
  ind[t]   = argmax_k dist[t,k]            (DVE tensor_tensor_reduce max + max_index)
  onehot   = (iota == ind)                 (ACT: square + relu trick, fp16)
  embed_sum/counts = onehot.T @ [x,1]      (fp16 PE matmuls accumulated in PSUM)
  quantize = embed[ind]                    (indirect DMA gather)

The per-cluster sums are all-reduced across the 8 shards on the host during the
gather/unshard step (as EnCodec's distributed all-reduce does), followed by the
tiny O(K*D) EMA update in fp32.

Note: argmax-of-distance problems have inherent fp32 tie sensitivity; this
kernel computes distances in fp32 on the PE (measured: 1 differing index out of
131072 vs the jax reference, same scale as any independent fp32 evaluation).
"""

import sys

sys.path.insert(0, "/opt/trn_rl_repo")

import numpy as np

import concourse.bass as bass
import concourse.tile as tile
from concourse import bacc, mybir
from concourse.bass import IndirectOffsetOnAxis, ts
from concourse.bass_utils import run_bass_kernel_spmd

F32 = mybir.dt.float32
F16 = mybir.dt.float16
U16 = mybir.dt.uint16
U32 = mybir.dt.uint32

D = 128
K = 1024
CORES = 8
TILE = 128
N_TOTAL = 32 * 4096
TOK_PER_CORE = N_TOTAL // CORES  # 16384
NT_FULL = TOK_PER_CORE // TILE  # 128 tiles per core

DECAY = 0.99
EPSILON = 1e-05
PREC = 10.0**7

NEG_HUGE = -3.0e38


def build_nc(n_tiles: int):
    """Build the per-core Bass program for `n_tiles` 128-token tiles."""
    T = n_tiles * TILE
    nc = bacc.Bacc(
        "TRN2",
        target_bir_lowering=False,
        debug=False,
        enable_asserts=False,
        num_devices=CORES,
    )

    xT_d = nc.dram_tensor("xT", [D, T], F32, kind="ExternalInput").ap()
    x16_d = nc.dram_tensor("x16", [T, D], F16, kind="ExternalInput").ap()
    e2_d = nc.dram_tensor("embT2", [D, K], F32, kind="ExternalInput").ap()
    emb_d = nc.dram_tensor("embed", [K, D], F32, kind="ExternalInput").ap()
    iota_d = nc.dram_tensor("iota", [D, K], U16, kind="ExternalInput").ap()

    q_d = nc.dram_tensor("quantize", [T, D], F32, kind="ExternalOutput").ap()
    ind_d = nc.dram_tensor("ind", [T, 1], U32, kind="ExternalOutput").ap()
    part_d = nc.dram_tensor("partial", [D, K], F32, kind="ExternalOutput").ap()

    with tile.TileContext(nc) as tc:
        with (
            tc.sbuf_pool(name="const", bufs=1) as cpool,
            tc.psum_pool(name="seg_ps", bufs=1) as segpool,
        ):
            # --- constants ---
            embT2 = cpool.tile([D, K], F32)
            nc.sync.dma_start(embT2[:], e2_d[:, :])
            iota_sb = cpool.tile([D, K], U16)
            nc.sync.dma_start(iota_sb[:], iota_d[:, :])

            ones_c = cpool.tile([D, 1], F32)
            nc.vector.memset(ones_c[:], 1.0)
            ones_r = cpool.tile([1, D], F32)
            nc.vector.memset(ones_r[:], 1.0)
            inmax8 = cpool.tile([D, 8], F32)
            nc.vector.memset(inmax8[:], NEG_HUGE)
            negind = cpool.tile([D, 1], F32)

            # --- -e_sq replicated across partitions ---
            # embT2 holds 2*e  ->  sum_d (2 e)^2 = 4*e_sq ; scale by -0.25.
            negesq = cpool.tile([D, K], F32)
            with tc.psum_pool(name="pre_ps", bufs=1) as prepool:
                sq2 = cpool.tile([D, K], F32)
                nc.vector.tensor_tensor(
                    out=sq2[:], in0=embT2[:], in1=embT2[:], op=mybir.AluOpType.mult
                )
                esq4_ps = prepool.tile([1, K], F32)
                for h in range(2):
                    nc.tensor.matmul(
                        out=esq4_ps[:, ts(h, 512)],
                        lhsT=ones_c[:],
                        rhs=sq2[:, ts(h, 512)],
                        start=True,
                        stop=True,
                    )
                esq_sb = cpool.tile([1, K], F32)
                nc.scalar.activation(
                    esq_sb[:], esq4_ps[:], mybir.ActivationFunctionType.Copy,
                    scale=-0.25,
                )
                rep_ps = prepool.tile([D, K], F32)
                for h in range(2):
                    nc.tensor.matmul(
                        out=rep_ps[:, ts(h, 512)],
                        lhsT=ones_r[:],
                        rhs=esq_sb[:, ts(h, 512)],
                        start=True,
                        stop=True,
                    )
                nc.vector.tensor_copy(out=negesq[:], in_=rep_ps[:])

            # --- per-cluster accumulator: embed_sum.T [D, K] over 2 PSUM banks ---
            esum_ps = segpool.tile([D, K], F32)

            with (
                tc.sbuf_pool(name="io", bufs=3) as io,
                tc.sbuf_pool(name="work", bufs=2) as work,
                tc.psum_pool(name="dist_ps", bufs=2) as dpool,
            ):
                for t in range(n_tiles):
                    xT_t = io.tile([D, TILE], F32)
                    nc.sync.dma_start(xT_t[:], xT_d[:, ts(t, TILE)])
                    x16_t = io.tile([TILE, D], F16)
                    nc.sync.dma_start(x16_t[:], x16_d[ts(t, TILE), :])

                    dist_ps = dpool.tile([TILE, K], F32)
                    for h in range(2):
                        nc.tensor.matmul(
                            out=dist_ps[:, ts(h, 512)],
                            lhsT=xT_t[:],
                            rhs=embT2[:, ts(h, 512)],
                            start=True,
                            stop=True,
                        )

                    # dist = cross2 - e_sq ; row max into inmax8[:,0]
                    # (InstTensorTensorReduce faults on this runtime; use two ops)
                    dist_sb = work.tile([TILE, K], F32)
                    nc.vector.tensor_tensor(
                        out=dist_sb[:],
                        in0=dist_ps[:],
                        in1=negesq[:],
                        op=mybir.AluOpType.add,
                    )
                    nc.vector.tensor_reduce(
                        out=inmax8[:, 0:1],
                        in_=dist_sb[:],
                        axis=mybir.AxisListType.X,
                        op=mybir.AluOpType.max,
                    )
                    ind8 = io.tile([TILE, 8], U32)
                    nc.vector.max_index(ind8[:], inmax8[:], dist_sb[:])

                    # one-hot(ind) in fp16 via ACT: relu(1 - |iota - ind|)
                    nc.scalar.activation(
                        negind[:], ind8[:, 0:1],
                        mybir.ActivationFunctionType.Copy, scale=-1.0,
                    )
                    sqd = work.tile([TILE, K], F16)
                    nc.scalar.activation(
                        sqd[:], iota_sb[:],
                        mybir.ActivationFunctionType.Abs, bias=negind[:, 0:1],
                    )
                    onehot = work.tile([TILE, K], F16)
                    nc.scalar.activation(
                        onehot[:], sqd[:],
                        mybir.ActivationFunctionType.Relu, bias=1.0, scale=-1.0,
                    )

                    # segment sums: esumT[d, k] += x[t, d] * onehot[t, k]
                    for h in range(2):
                        nc.tensor.matmul(
                            out=esum_ps[:, ts(h, 512)],
                            lhsT=x16_t[:],
                            rhs=onehot[:, ts(h, 512)],
                            start=(t == 0),
                            stop=(t == n_tiles - 1),
                        )

                    # quantize = embed[ind]
                    q_t = io.tile([TILE, D], F32)
                    nc.gpsimd.indirect_dma_start(
                        out=q_t[:],
                        out_offset=None,
                        in_=emb_d[:, :],
                        in_offset=IndirectOffsetOnAxis(ap=ind8[:, 0:1], axis=0),
                    )
                    nc.sync.dma_start(q_d[ts(t, TILE), :], q_t[:])
                    nc.sync.dma_start(ind_d[ts(t, TILE), :], ind8[:, 0:1])

            # --- flush per-cluster accumulator ---
            seg_sb = cpool.tile([D, K], F32, name="segsb")
            nc.scalar.copy(seg_sb[:], esum_ps[:])
            nc.sync.dma_start(part_d[:, :], seg_sb[:])

    nc.compile()
    return nc


_NC_CACHE: dict[int, object] = {}


def _get_nc(n_tiles: int):
    if n_tiles not in _NC_CACHE:
        _NC_CACHE[n_tiles] = build_nc(n_tiles)
    return _NC_CACHE[n_tiles]


def _qt32(t: np.ndarray) -> np.ndarray:
    p = np.float32(PREC)
    return (np.round(t * p) / p).astype(np.float32)


def make_in_maps(x: np.ndarray, embed: np.ndarray, n_tiles: int = NT_FULL):
    """Shard inputs for the 8 cores."""
    tok = n_tiles * TILE
    flat = np.ascontiguousarray(x.reshape(-1, D).astype(np.float32, copy=False))
    embed = np.asarray(embed, dtype=np.float32)
    embT2 = np.ascontiguousarray((2.0 * _qt32(embed)).T.astype(np.float32))
    iota = np.ascontiguousarray(
        np.broadcast_to(np.arange(K, dtype=np.uint16), (D, K))
    )
    in_maps = []
    for c in range(CORES):
        shard = flat[c * tok : (c + 1) * tok]
        in_maps.append(
            {
                "xT": np.ascontiguousarray(shard.T),
                "x16": shard.astype(np.float16),
                "embT2": embT2,
                "embed": embed,
                "iota": iota,
            }
        )
    return in_maps


def ema_tail(counts, embed_sum, cluster_size, embed_avg):
    """The tiny O(K*D) EMA update, fp32 exactly as the reference."""
    one = np.float32(1.0)
    decay = np.float32(DECAY)
    omd = np.float32(1.0 - DECAY)
    counts = counts.astype(np.float32)
    embed_sum = embed_sum.astype(np.float32)
    new_cluster_size = cluster_size * decay + omd * counts
    new_embed_avg = embed_avg * decay + omd * embed_sum
    total = new_cluster_size.sum(dtype=np.float32)
    eps = np.float32(EPSILON)
    keps = np.float32(K * EPSILON)
    smoothed = (new_cluster_size + eps) / (total + keps) * total
    new_embed = new_embed_avg / smoothed[:, None]
    return new_cluster_size, new_embed_avg, new_embed


def run_cores(x, embed, n_tiles: int = NT_FULL, trace: bool = False, **kw):
    nc = _get_nc(n_tiles)
    in_maps = make_in_maps(x, embed, n_tiles)
    res = run_bass_kernel_spmd(
        nc, in_maps, core_ids=list(range(CORES)), trace=trace, **kw
    )
    return res


def kernel(x, embed, cluster_size, embed_avg):
    x = np.asarray(x, dtype=np.float32)
    embed = np.asarray(embed, dtype=np.float32)
    cluster_size = np.asarray(cluster_size, dtype=np.float32)
    embed_avg = np.asarray(embed_avg, dtype=np.float32)

    res = run_cores(x, embed)
    outs = res.results

    quantize = np.concatenate([o["quantize"] for o in outs], axis=0)
    quantize = quantize.reshape(x.shape)
    ind = np.concatenate([o["ind"] for o in outs], axis=0)[:, 0]
    embed_ind = ind.view(np.int32).reshape(x.shape[:-1])

    partial = np.zeros((D, K), dtype=np.float32)
    for o in outs:
        partial += o["partial"]
    embed_sum = np.ascontiguousarray(partial.T)
    counts = np.bincount(ind.view(np.int32), minlength=K).astype(np.float32)

    new_cluster_size, new_embed_avg, new_embed = ema_tail(
        counts, embed_sum, cluster_size, embed_avg
    )
    return quantize, embed_ind, new_cluster_size, new_embed_avg, new_embed


# revision 6
# speedup vs baseline: 1.0207x; 1.0207x over previous
"""Trainium2 Bass kernel for nn_EuclideanCodebook (EnCodec VQ codebook, training step).

Data-parallel over 8 NeuronCores: flattened tokens N=32*4096=131072 are sharded
128 tiles/core x 128 tokens; embed (1024x128) is replicated. Per core:

  dist[t,k] = 2*x_t.e_k - |e_k|^2          (fp32 PE matmul + fused DVE subtract)
  ind[t]   = argmax_k dist[t,k]            (DVE tensor_tensor_reduce max + max_index)
  onehot   = (iota == ind)                 (ACT: square + relu trick, fp16)
  embed_sum/counts = onehot.T @ [x,1]      (fp16 PE matmuls accumulated in PSUM)
  quantize = embed[ind]                    (indirect DMA gather)

The per-cluster sums are all-reduced across the 8 shards on the host during the
gather/unshard step (as EnCodec's distributed all-reduce does), followed by the
tiny O(K*D) EMA update in fp32.

Note: argmax-of-distance problems have inherent fp32 tie sensitivity; this
kernel computes distances in fp32 on the PE (measured: 1 differing index out of
131072 vs the jax reference, same scale as any independent fp32 evaluation).
"""

import sys

sys.path.insert(0, "/opt/trn_rl_repo")

import numpy as np

import concourse.bass as bass
import concourse.tile as tile
from concourse import bacc, mybir
from concourse.bass import IndirectOffsetOnAxis, ts
from concourse.bass_utils import run_bass_kernel_spmd

F32 = mybir.dt.float32
F16 = mybir.dt.float16
U16 = mybir.dt.uint16
U32 = mybir.dt.uint32

D = 128
K = 1024
CORES = 8
TILE = 128
N_TOTAL = 32 * 4096
TOK_PER_CORE = N_TOTAL // CORES  # 16384
NT_FULL = TOK_PER_CORE // TILE  # 128 tiles per core

DECAY = 0.99
EPSILON = 1e-05
PREC = 10.0**7

NEG_HUGE = -3.0e38


def build_nc(n_tiles: int):
    """Build the per-core Bass program for `n_tiles` 128-token tiles."""
    T = n_tiles * TILE
    nc = bacc.Bacc(
        "TRN2",
        target_bir_lowering=False,
        debug=False,
        enable_asserts=False,
        num_devices=CORES,
    )

    xT_d = nc.dram_tensor("xT", [D, T], F32, kind="ExternalInput").ap()
    x16_d = nc.dram_tensor("x16", [T, D], F16, kind="ExternalInput").ap()
    e2_d = nc.dram_tensor("embT2", [D, K], F32, kind="ExternalInput").ap()
    emb_d = nc.dram_tensor("embed", [K, D], F32, kind="ExternalInput").ap()
    iota_d = nc.dram_tensor("iota", [D, K], U16, kind="ExternalInput").ap()

    q_d = nc.dram_tensor("quantize", [T, D], F32, kind="ExternalOutput").ap()
    ind_d = nc.dram_tensor("ind", [T, 1], U32, kind="ExternalOutput").ap()
    part_d = nc.dram_tensor("partial", [D, K], F32, kind="ExternalOutput").ap()

    with tile.TileContext(nc) as tc:
        with (
            tc.sbuf_pool(name="const", bufs=1) as cpool,
            tc.psum_pool(name="seg_ps", bufs=1) as segpool,
        ):
            # --- constants ---
            embT2 = cpool.tile([D, K], F32)
            nc.sync.dma_start(embT2[:], e2_d[:, :])
            iota_sb = cpool.tile([D, K], U16)
            nc.sync.dma_start(iota_sb[:], iota_d[:, :])

            ones_c = cpool.tile([D, 1], F32)
            nc.vector.memset(ones_c[:], 1.0)
            ones_r = cpool.tile([1, D], F32)
            nc.vector.memset(ones_r[:], 1.0)
            inmax8 = cpool.tile([D, 8], F32)
            nc.vector.memset(inmax8[:], NEG_HUGE)
            negind = cpool.tile([D, 1], F32)

            # --- -e_sq replicated across partitions ---
            # embT2 holds 2*e  ->  sum_d (2 e)^2 = 4*e_sq ; scale by -0.25.
            negesq = cpool.tile([D, K], F32)
            with tc.psum_pool(name="pre_ps", bufs=1) as prepool:
                sq2 = cpool.tile([D, K], F32)
                nc.vector.tensor_tensor(
                    out=sq2[:], in0=embT2[:], in1=embT2[:], op=mybir.AluOpType.mult
                )
                esq4_ps = prepool.tile([1, K], F32)
                for h in range(2):
                    nc.tensor.matmul(
                        out=esq4_ps[:, ts(h, 512)],
                        lhsT=ones_c[:],
                        rhs=sq2[:, ts(h, 512)],
                        start=True,
                        stop=True,
                    )
                esq_sb = cpool.tile([1, K], F32)
                nc.scalar.activation(
                    esq_sb[:], esq4_ps[:], mybir.ActivationFunctionType.Copy,
                    scale=-0.25,
                )
                rep_ps = prepool.tile([D, K], F32)
                for h in range(2):
                    nc.tensor.matmul(
                        out=rep_ps[:, ts(h, 512)],
                        lhsT=ones_r[:],
                        rhs=esq_sb[:, ts(h, 512)],
                        start=True,
                        stop=True,
                    )
                nc.vector.tensor_copy(out=negesq[:], in_=rep_ps[:])

            # --- per-cluster accumulator: embed_sum.T [D, K] over 2 PSUM banks ---
            esum_ps = segpool.tile([D, K], F32)

            with (
                tc.sbuf_pool(name="io", bufs=4) as io,
                tc.sbuf_pool(name="work", bufs=3) as work,
                tc.psum_pool(name="dist_ps", bufs=3) as dpool,
            ):
                pend = []
                seg_first = True
                for t in range(n_tiles):
                    xT_t = io.tile([D, TILE], F32)
                    nc.sync.dma_start(xT_t[:], xT_d[:, ts(t, TILE)])
                    x16_t = io.tile([TILE, D], F16)
                    nc.sync.dma_start(x16_t[:], x16_d[ts(t, TILE), :])

                    dist_ps = dpool.tile([TILE, K], F32)
                    for h in range(2):
                        nc.tensor.matmul(
                            out=dist_ps[:, ts(h, 512)],
                            lhsT=xT_t[:],
                            rhs=embT2[:, ts(h, 512)],
                            start=True,
                            stop=True,
                        )

                    # dist = cross2 - e_sq ; row max into inmax8[:,0]
                    # (InstTensorTensorReduce faults on this runtime; use two ops)
                    dist_sb = work.tile([TILE, K], F32)
                    nc.vector.tensor_tensor(
                        out=dist_sb[:],
                        in0=dist_ps[:],
                        in1=negesq[:],
                        op=mybir.AluOpType.add,
                    )
                    nc.vector.tensor_reduce(
                        out=inmax8[:, 0:1],
                        in_=dist_sb[:],
                        axis=mybir.AxisListType.X,
                        op=mybir.AluOpType.max,
                    )
                    ind8 = io.tile([TILE, 8], U32)
                    nc.vector.max_index(ind8[:], inmax8[:], dist_sb[:])

                    # one-hot(ind) in fp16 via ACT: relu(1 - |iota - ind|)
                    nc.scalar.activation(
                        negind[:], ind8[:, 0:1],
                        mybir.ActivationFunctionType.Copy, scale=-1.0,
                    )
                    sqd = work.tile([TILE, K], F16)
                    nc.scalar.activation(
                        sqd[:], iota_sb[:],
                        mybir.ActivationFunctionType.Abs, bias=negind[:, 0:1],
                    )
                    onehot = work.tile([TILE, K], F16)
                    nc.scalar.activation(
                        onehot[:], sqd[:],
                        mybir.ActivationFunctionType.Relu, bias=1.0, scale=-1.0,
                    )

                    # segment sums: esumT[d, k] += x[t, d] * onehot[t, k]
                    # (delayed one tile so PE never waits on this tile's
                    #  argmax -> one-hot chain)
                    pend.append((x16_t, onehot))
                    if len(pend) == 2:
                        px, po = pend.pop(0)
                        for h in range(2):
                            nc.tensor.matmul(
                                out=esum_ps[:, ts(h, 512)],
                                lhsT=px[:],
                                rhs=po[:, ts(h, 512)],
                                start=seg_first,
                                stop=False,
                            )
                        seg_first = False

                    # quantize = embed[ind]
                    q_t = io.tile([TILE, D], F32)
                    nc.gpsimd.indirect_dma_start(
                        out=q_t[:],
                        out_offset=None,
                        in_=emb_d[:, :],
                        in_offset=IndirectOffsetOnAxis(ap=ind8[:, 0:1], axis=0),
                    )
                    nc.sync.dma_start(q_d[ts(t, TILE), :], q_t[:])
                    nc.sync.dma_start(ind_d[ts(t, TILE), :], ind8[:, 0:1])

                # drain the delayed segment-sum matmul
                px, po = pend.pop(0)
                for h in range(2):
                    nc.tensor.matmul(
                        out=esum_ps[:, ts(h, 512)],
                        lhsT=px[:],
                        rhs=po[:, ts(h, 512)],
                        start=seg_first,
                        stop=True,
                    )

            # --- flush per-cluster accumulator ---
            seg_sb = cpool.tile([D, K], F32, name="segsb")
            nc.scalar.copy(seg_sb[:], esum_ps[:])
            nc.sync.dma_start(part_d[:, :], seg_sb[:])

    nc.compile()
    return nc


_NC_CACHE: dict[int, object] = {}


def _get_nc(n_tiles: int):
    if n_tiles not in _NC_CACHE:
        _NC_CACHE[n_tiles] = build_nc(n_tiles)
    return _NC_CACHE[n_tiles]


def _qt32(t: np.ndarray) -> np.ndarray:
    p = np.float32(PREC)
    return (np.round(t * p) / p).astype(np.float32)


def make_in_maps(x: np.ndarray, embed: np.ndarray, n_tiles: int = NT_FULL):
    """Shard inputs for the 8 cores."""
    tok = n_tiles * TILE
    flat = np.ascontiguousarray(x.reshape(-1, D).astype(np.float32, copy=False))
    embed = np.asarray(embed, dtype=np.float32)
    embT2 = np.ascontiguousarray((2.0 * _qt32(embed)).T.astype(np.float32))
    iota = np.ascontiguousarray(
        np.broadcast_to(np.arange(K, dtype=np.uint16), (D, K))
    )
    in_maps = []
    for c in range(CORES):
        shard = flat[c * tok : (c + 1) * tok]
        in_maps.append(
            {
                "xT": np.ascontiguousarray(shard.T),
                "x16": shard.astype(np.float16),
                "embT2": embT2,
                "embed": embed,
                "iota": iota,
            }
        )
    return in_maps


def ema_tail(counts, embed_sum, cluster_size, embed_avg):
    """The tiny O(K*D) EMA update, fp32 exactly as the reference."""
    one = np.float32(1.0)
    decay = np.float32(DECAY)
    omd = np.float32(1.0 - DECAY)
    counts = counts.astype(np.float32)
    embed_sum = embed_sum.astype(np.float32)
    new_cluster_size = cluster_size * decay + omd * counts
    new_embed_avg = embed_avg * decay + omd * embed_sum
    total = new_cluster_size.sum(dtype=np.float32)
    eps = np.float32(EPSILON)
    keps = np.float32(K * EPSILON)
    smoothed = (new_cluster_size + eps) / (total + keps) * total
    new_embed = new_embed_avg / smoothed[:, None]
    return new_cluster_size, new_embed_avg, new_embed


def run_cores(x, embed, n_tiles: int = NT_FULL, trace: bool = False, **kw):
    nc = _get_nc(n_tiles)
    in_maps = make_in_maps(x, embed, n_tiles)
    res = run_bass_kernel_spmd(
        nc, in_maps, core_ids=list(range(CORES)), trace=trace, **kw
    )
    return res


def kernel(x, embed, cluster_size, embed_avg):
    x = np.asarray(x, dtype=np.float32)
    embed = np.asarray(embed, dtype=np.float32)
    cluster_size = np.asarray(cluster_size, dtype=np.float32)
    embed_avg = np.asarray(embed_avg, dtype=np.float32)

    res = run_cores(x, embed)
    outs = res.results

    quantize = np.concatenate([o["quantize"] for o in outs], axis=0)
    quantize = quantize.reshape(x.shape)
    ind = np.concatenate([o["ind"] for o in outs], axis=0)[:, 0]
    embed_ind = ind.view(np.int32).reshape(x.shape[:-1])

    partial = np.zeros((D, K), dtype=np.float32)
    for o in outs:
        partial += o["partial"]
    embed_sum = np.ascontiguousarray(partial.T)
    counts = np.bincount(ind.view(np.int32), minlength=K).astype(np.float32)

    new_cluster_size, new_embed_avg, new_embed = ema_tail(
        counts, embed_sum, cluster_size, embed_avg
    )
    return quantize, embed_ind, new_cluster_size, new_embed_avg, new_embed


# revision 9
# speedup vs baseline: 1.0347x; 1.0138x over previous
"""Trainium2 Bass kernel for nn_EuclideanCodebook (EnCodec VQ codebook, training step).

Data-parallel over 8 NeuronCores: flattened tokens N=32*4096=131072 are sharded
128 tiles/core x 128 tokens; embed (1024x128) is replicated. Per core:

  dist[t,k] = 2*x_t.e_k - |e_k|^2          (fp32 PE matmul + fused DVE subtract)
  ind[t]   = argmax_k dist[t,k]            (DVE tensor_tensor_reduce max + max_index)
  onehot   = (iota == ind)                 (ACT: square + relu trick, fp16)
  embed_sum/counts = onehot.T @ [x,1]      (fp16 PE matmuls accumulated in PSUM)
  quantize = embed[ind]                    (indirect DMA gather)

The per-cluster sums are all-reduced across the 8 shards on the host during the
gather/unshard step (as EnCodec's distributed all-reduce does), followed by the
tiny O(K*D) EMA update in fp32.

Note: argmax-of-distance problems have inherent fp32 tie sensitivity; this
kernel computes distances in fp32 on the PE (measured: 1 differing index out of
131072 vs the jax reference, same scale as any independent fp32 evaluation).
"""

import sys

sys.path.insert(0, "/opt/trn_rl_repo")

import numpy as np

import concourse.bass as bass
import concourse.tile as tile
from concourse import bacc, library_config, mybir
from concourse.bass import IndirectOffsetOnAxis, ts
from concourse.bass_utils import run_bass_kernel_spmd

F32 = mybir.dt.float32
F16 = mybir.dt.float16
U16 = mybir.dt.uint16
I16 = mybir.dt.int16
U32 = mybir.dt.uint32

D = 128
K = 1024
CORES = 8
TILE = 128
N_TOTAL = 32 * 4096
TOK_PER_CORE = N_TOTAL // CORES  # 16384
NT_FULL = TOK_PER_CORE // TILE  # 128 tiles per core

DECAY = 0.99
EPSILON = 1e-05
PREC = 10.0**7

NEG_HUGE = -3.0e38


def build_nc(n_tiles: int):
    """Build the per-core Bass program for `n_tiles` 128-token tiles."""
    T = n_tiles * TILE
    nc = bacc.Bacc(
        "TRN2",
        target_bir_lowering=False,
        debug=False,
        enable_asserts=False,
        num_devices=CORES,
    )

    xT_d = nc.dram_tensor("xT", [D, T], F32, kind="ExternalInput").ap()
    x16_d = nc.dram_tensor("x16", [T, D], F16, kind="ExternalInput").ap()
    e2_d = nc.dram_tensor("embT2", [D, K], F32, kind="ExternalInput").ap()
    emb_d = nc.dram_tensor("embed", [K, D], F32, kind="ExternalInput").ap()

    q_d = nc.dram_tensor("quantize", [T, D], F32, kind="ExternalOutput").ap()
    ind_d = nc.dram_tensor("ind", [T, 1], U32, kind="ExternalOutput").ap()
    part_d = nc.dram_tensor("partial", [D, K], F32, kind="ExternalOutput").ap()

    with tile.TileContext(nc) as tc:
        with (
            tc.sbuf_pool(name="const", bufs=1) as cpool,
            tc.psum_pool(name="seg_ps", bufs=1) as segpool,
        ):
            # --- constants ---
            embT2 = cpool.tile([D, K], F32)
            nc.sync.dma_start(embT2[:], e2_d[:, :])
            scat_dat = cpool.tile([D, 2], F16)
            nc.vector.memset(scat_dat[:, 0:1], 1.0)
            nc.vector.memset(scat_dat[:, 1:2], 0.0)
            nc.gpsimd.load_library(library_config.local_scatter)

            ones_c = cpool.tile([D, 1], F32)
            nc.vector.memset(ones_c[:], 1.0)
            ones_r = cpool.tile([1, D], F32)
            nc.vector.memset(ones_r[:], 1.0)
            inmax8 = cpool.tile([D, 8], F32)
            nc.vector.memset(inmax8[:], NEG_HUGE)
            negone = cpool.tile([D, 1], F32)
            nc.vector.memset(negone[:], -1.0)

            # --- -e_sq replicated across partitions ---
            # embT2 holds 2*e  ->  sum_d (2 e)^2 = 4*e_sq ; scale by -0.25.
            negesq = cpool.tile([D, K], F32)
            with tc.psum_pool(name="pre_ps", bufs=1) as prepool:
                sq2 = cpool.tile([D, K], F32)
                nc.vector.tensor_tensor(
                    out=sq2[:], in0=embT2[:], in1=embT2[:], op=mybir.AluOpType.mult
                )
                esq4_ps = prepool.tile([1, K], F32)
                for h in range(2):
                    nc.tensor.matmul(
                        out=esq4_ps[:, ts(h, 512)],
                        lhsT=ones_c[:],
                        rhs=sq2[:, ts(h, 512)],
                        start=True,
                        stop=True,
                    )
                esq_sb = cpool.tile([1, K], F32)
                nc.scalar.activation(
                    esq_sb[:], esq4_ps[:], mybir.ActivationFunctionType.Copy,
                    scale=-0.25,
                )
                rep_ps = prepool.tile([D, K], F32)
                for h in range(2):
                    nc.tensor.matmul(
                        out=rep_ps[:, ts(h, 512)],
                        lhsT=ones_r[:],
                        rhs=esq_sb[:, ts(h, 512)],
                        start=True,
                        stop=True,
                    )
                nc.vector.tensor_copy(out=negesq[:], in_=rep_ps[:])

            # --- per-cluster accumulator: embed_sum.T [D, K] over 2 PSUM banks ---
            esum_ps = segpool.tile([D, K], F32)

            with (
                tc.sbuf_pool(name="io", bufs=4) as io,
                tc.sbuf_pool(name="work", bufs=3) as work,
                tc.psum_pool(name="dist_ps", bufs=3) as dpool,
            ):
                pend = []
                seg_first = True
                for t in range(n_tiles):
                    xT_t = io.tile([D, TILE], F32)
                    nc.sync.dma_start(xT_t[:], xT_d[:, ts(t, TILE)])
                    x16_t = io.tile([TILE, D], F16)
                    nc.sync.dma_start(x16_t[:], x16_d[ts(t, TILE), :])

                    dist_ps = dpool.tile([TILE, K], F32)
                    for h in range(2):
                        nc.tensor.matmul(
                            out=dist_ps[:, ts(h, 512)],
                            lhsT=xT_t[:],
                            rhs=embT2[:, ts(h, 512)],
                            start=True,
                            stop=True,
                        )

                    # dist = cross2 - e_sq ; row max into inmax8[:,0]
                    # (InstTensorTensorReduce faults on this runtime; use two ops)
                    dist_sb = work.tile([TILE, K], F32)
                    nc.vector.tensor_tensor(
                        out=dist_sb[:],
                        in0=dist_ps[:],
                        in1=negesq[:],
                        op=mybir.AluOpType.add,
                    )
                    nc.vector.tensor_reduce(
                        out=inmax8[:, 0:1],
                        in_=dist_sb[:],
                        axis=mybir.AxisListType.X,
                        op=mybir.AluOpType.max,
                    )
                    ind8 = io.tile([TILE, 8], U32)
                    nc.vector.max_index(ind8[:], inmax8[:], dist_sb[:])

                    # one-hot(ind) in fp16 on GPSIMD: dst[p, :]=0; dst[p, idx[p, j]] = data[p, j]
                    # idxs = [ind, ind-1] int16, data = [1, 0]; the second slot
                    # only pads num_idxs to an even count (writes 0.0, and a
                    # negative index when ind==0 is ignored by the op)
                    idx16 = io.tile([TILE, 2], I16)
                    nc.scalar.activation(
                        idx16[:, 0:1], ind8[:, 0:1],
                        mybir.ActivationFunctionType.Copy,
                    )
                    nc.scalar.activation(
                        idx16[:, 1:2], ind8[:, 0:1],
                        mybir.ActivationFunctionType.Identity, bias=negone[:, 0:1],
                    )
                    onehot = work.tile([TILE, K], F16)
                    nc.gpsimd.local_scatter(
                        out_ap=onehot[:],
                        data_ap=scat_dat[:],
                        idxs_ap=idx16[:],
                        channels=TILE,
                        num_elems=K,
                        num_idxs=2,
                    )

                    # segment sums: esumT[d, k] += x[t, d] * onehot[t, k]
                    # (delayed one tile so PE never waits on this tile's
                    #  argmax -> one-hot chain)
                    pend.append((x16_t, onehot))
                    if len(pend) == 2:
                        px, po = pend.pop(0)
                        for h in range(2):
                            nc.tensor.matmul(
                                out=esum_ps[:, ts(h, 512)],
                                lhsT=px[:],
                                rhs=po[:, ts(h, 512)],
                                start=seg_first,
                                stop=False,
                            )
                        seg_first = False

                    # quantize = embed[ind]
                    q_t = io.tile([TILE, D], F32)
                    nc.gpsimd.indirect_dma_start(
                        out=q_t[:],
                        out_offset=None,
                        in_=emb_d[:, :],
                        in_offset=IndirectOffsetOnAxis(ap=ind8[:, 0:1], axis=0),
                    )
                    nc.sync.dma_start(q_d[ts(t, TILE), :], q_t[:])
                    nc.sync.dma_start(ind_d[ts(t, TILE), :], ind8[:, 0:1])

                # drain the delayed segment-sum matmul
                px, po = pend.pop(0)
                for h in range(2):
                    nc.tensor.matmul(
                        out=esum_ps[:, ts(h, 512)],
                        lhsT=px[:],
                        rhs=po[:, ts(h, 512)],
                        start=seg_first,
                        stop=True,
                    )

            # --- flush per-cluster accumulator ---
            seg_sb = cpool.tile([D, K], F32, name="segsb")
            nc.scalar.copy(seg_sb[:], esum_ps[:])
            nc.sync.dma_start(part_d[:, :], seg_sb[:])

    nc.compile()
    return nc


_NC_CACHE: dict[int, object] = {}


def _get_nc(n_tiles: int):
    if n_tiles not in _NC_CACHE:
        _NC_CACHE[n_tiles] = build_nc(n_tiles)
    return _NC_CACHE[n_tiles]


def _qt32(t: np.ndarray) -> np.ndarray:
    p = np.float32(PREC)
    return (np.round(t * p) / p).astype(np.float32)


def make_in_maps(x: np.ndarray, embed: np.ndarray, n_tiles: int = NT_FULL):
    """Shard inputs for the 8 cores."""
    tok = n_tiles * TILE
    flat = np.ascontiguousarray(x.reshape(-1, D).astype(np.float32, copy=False))
    embed = np.asarray(embed, dtype=np.float32)
    embT2 = np.ascontiguousarray((2.0 * _qt32(embed)).T.astype(np.float32))
    in_maps = []
    for c in range(CORES):
        shard = flat[c * tok : (c + 1) * tok]
        in_maps.append(
            {
                "xT": np.ascontiguousarray(shard.T),
                "x16": shard.astype(np.float16),
                "embT2": embT2,
                "embed": embed,
            }
        )
    return in_maps


def ema_tail(counts, embed_sum, cluster_size, embed_avg):
    """The tiny O(K*D) EMA update, fp32 exactly as the reference."""
    one = np.float32(1.0)
    decay = np.float32(DECAY)
    omd = np.float32(1.0 - DECAY)
    counts = counts.astype(np.float32)
    embed_sum = embed_sum.astype(np.float32)
    new_cluster_size = cluster_size * decay + omd * counts
    new_embed_avg = embed_avg * decay + omd * embed_sum
    total = new_cluster_size.sum(dtype=np.float32)
    eps = np.float32(EPSILON)
    keps = np.float32(K * EPSILON)
    smoothed = (new_cluster_size + eps) / (total + keps) * total
    new_embed = new_embed_avg / smoothed[:, None]
    return new_cluster_size, new_embed_avg, new_embed


def run_cores(x, embed, n_tiles: int = NT_FULL, trace: bool = False, **kw):
    nc = _get_nc(n_tiles)
    in_maps = make_in_maps(x, embed, n_tiles)
    res = run_bass_kernel_spmd(
        nc, in_maps, core_ids=list(range(CORES)), trace=trace, **kw
    )
    return res


def kernel(x, embed, cluster_size, embed_avg):
    x = np.asarray(x, dtype=np.float32)
    embed = np.asarray(embed, dtype=np.float32)
    cluster_size = np.asarray(cluster_size, dtype=np.float32)
    embed_avg = np.asarray(embed_avg, dtype=np.float32)

    res = run_cores(x, embed)
    outs = res.results

    quantize = np.concatenate([o["quantize"] for o in outs], axis=0)
    quantize = quantize.reshape(x.shape)
    ind = np.concatenate([o["ind"] for o in outs], axis=0)[:, 0]
    embed_ind = ind.view(np.int32).reshape(x.shape[:-1])

    partial = np.zeros((D, K), dtype=np.float32)
    for o in outs:
        partial += o["partial"]
    embed_sum = np.ascontiguousarray(partial.T)
    counts = np.bincount(ind.view(np.int32), minlength=K).astype(np.float32)

    new_cluster_size, new_embed_avg, new_embed = ema_tail(
        counts, embed_sum, cluster_size, embed_avg
    )
    return quantize, embed_ind, new_cluster_size, new_embed_avg, new_embed
